# revision 6
# baseline (speedup 1.0000x reference)
"""Trainium2 Bass kernel for CodecNormalizer (retrieval_knn).

Pipeline (per reference):
  d_emb = sv_embed(dysarthric_codec)   # [16, 256]   (mean over T, MLP, L2 norm)
  n_emb = sv_embed(normal_codec_set)   # [4096, 256]
  dist  = L1(d_emb, n_emb)             # [16, 4096]
  out   = normal_codec_set[argmin(dist, axis=1)]

Sharding: normal_codec_set is split along N across 8 cores (512 codecs each);
the dysarthric batch is replicated.  Each core computes its [16, 512] slice of
the distance matrix on-device; the host does the final argmin over 4096
candidates and gathers the winning codecs from the (host-resident) input.

Device kernel layout notes:
  - The big cost is streaming the 64MB/core normal shard.  Tiles are
    [128 part, 16 rows, 128 d] f32 = 1MB of contiguous DRAM.
  - Mean over Tn=256 uses the tensor engine with the *data as the stationary
    operand* (LDWEIGHTS streams at 1 col/cycle regardless of dtype) and a tiny
    block-diagonal ones/256 matrix as the moving operand: fp32 matmuls pay a
    4x penalty only on moving-operand rows, which is just 8 columns here.
  - MLP keeps features on partitions / codecs on the free axis so layers chain
    without transposes: out[h_out, c] = W[h_in, h_out].T @ act[h_in, c].
  - L2 norm: Square (ACT) -> ones-matmul partition reduction (PE) -> Sqrt
    (ACT) -> reciprocal (DVE) -> outer-product broadcast (PE) -> multiply.
  - L1 distance: per query b, |n_emb - d_emb[:, b]| is one fused DVE
    tensor_scalar (op0=subtract, op1=abs_max vs 0.0), then a ones-matmul sums
    over the 128 embedding partitions into psum row [1, C].
"""

import numpy as np

# Problem shapes (hardcoded per contract).
B, T, D = 16, 512, 128
N, TN = 4096, 256
E, H = 256, 512
N_CORES = 8
NSH = N // N_CORES  # codecs per core

F = 16  # rows per partition in a stream tile
TILE_ROWS = 128 * F  # 2048 rows = 8 codecs (normal) / 4 items (dys) per tile
CODECS_PER_TILE = TILE_ROWS // TN  # 8
DYS_PER_TILE = TILE_ROWS // T  # 4

# Number of leading stream tiles in each group whose f-axis reduction is done
# on the vector engine (tree add) instead of 16 PE ldweights+matmul pairs.
# Balances PE vs DVE occupancy; both must stay under the DMA roofline.
DVE_TILES_PER_GROUP = 10

_BUILD_CACHE: dict = {}


def _build(nsh):
    import concourse.bacc as bacc
    import concourse.tile as tile
    from concourse import mybir
    from concourse.mybir import AluOpType as alu
    from concourse.mybir import ActivationFunctionType as act
    from contextlib import ExitStack

    f32 = mybir.dt.float32

    n_tiles = nsh * TN // TILE_ROWS
    group_tiles = min(16, n_tiles)
    groups = n_tiles // group_tiles
    group_c = group_tiles * CODECS_PER_TILE  # codecs per group (<=128)
    assert groups * group_tiles == n_tiles and group_c <= 128

    nc = bacc.Bacc("TRN2", target_bir_lowering=False, debug=False)

    normal = nc.dram_tensor("normal", [nsh, TN, D], f32, kind="ExternalInput")
    dys = nc.dram_tensor("dys", [B, T, D], f32, kind="ExternalInput")
    w1 = nc.dram_tensor("w1", [D, H], f32, kind="ExternalInput")
    b1 = nc.dram_tensor("b1", [H], f32, kind="ExternalInput")
    w2 = nc.dram_tensor("w2", [H, H], f32, kind="ExternalInput")
    b2 = nc.dram_tensor("b2", [H], f32, kind="ExternalInput")
    w3 = nc.dram_tensor("w3", [H, E], f32, kind="ExternalInput")
    b3 = nc.dram_tensor("b3", [E], f32, kind="ExternalInput")
    # Block-diagonal scaled-ones matrices for the mean-pool matmuls.
    blk_n = nc.dram_tensor("blk_n", [128, CODECS_PER_TILE], f32, kind="ExternalInput")
    blk_d = nc.dram_tensor("blk_d", [128, DYS_PER_TILE], f32, kind="ExternalInput")
    dist = nc.dram_tensor("dist", [B, nsh], f32, kind="ExternalOutput")

    # Tiled DRAM views: tile TT covers 2048 consecutive rows; partition p holds
    # rows [16p, 16p+16) of the tile, so each partition is an 8KB contiguous
    # chunk and the whole tile is one contiguous 1MB block.
    normal_t = normal.ap().rearrange(
        "(TT phi) (plo f) d -> TT (phi plo) f d",
        phi=CODECS_PER_TILE, plo=128 // CODECS_PER_TILE, f=F,
    )
    dys_t = dys.ap().rearrange(
        "(TT phi) (plo f) d -> TT (phi plo) f d",
        phi=DYS_PER_TILE, plo=128 // DYS_PER_TILE, f=F,
    )

    with ExitStack() as ctx:
        tc = ctx.enter_context(tile.TileContext(nc))
        singles = ctx.enter_context(tc.tile_pool(name="singles", bufs=1))
        stream = ctx.enter_context(tc.tile_pool(name="stream", bufs=6))
        sb = ctx.enter_context(tc.tile_pool(name="sb", bufs=2))
        tmp_pool = ctx.enter_context(tc.tile_pool(name="tmp", bufs=4))
        dout_pool = ctx.enter_context(tc.tile_pool(name="dout", bufs=4))
        pacc = ctx.enter_context(tc.tile_pool(name="pacc", bufs=2, space="PSUM"))
        pmlp = ctx.enter_context(tc.tile_pool(name="pmlp", bufs=2, space="PSUM"))
        pvec = ctx.enter_context(tc.tile_pool(name="pvec", bufs=2, space="PSUM"))
        pdist = ctx.enter_context(tc.tile_pool(name="pdist", bufs=2, space="PSUM"))

        # ---- constants ----
        w1_sb = singles.tile([D, H], f32)
        nc.sync.dma_start(w1_sb, w1.ap())
        w2_sb = singles.tile([128, 4, H], f32)
        nc.sync.dma_start(w2_sb, w2.ap().rearrange("(kc p) h -> p kc h", p=128))
        w3_sb = singles.tile([128, 4, E], f32)
        nc.sync.dma_start(w3_sb, w3.ap().rearrange("(kc p) e -> p kc e", p=128))
        b1_sb = singles.tile([128, 4], f32)
        nc.sync.dma_start(b1_sb, b1.ap().rearrange("(mc p) -> p mc", p=128))
        b2_sb = singles.tile([128, 4], f32)
        nc.sync.dma_start(b2_sb, b2.ap().rearrange("(mc p) -> p mc", p=128))
        b3_sb = singles.tile([128, 2], f32)
        nc.sync.dma_start(b3_sb, b3.ap().rearrange("(ec p) -> p ec", p=128))
        blkn_sb = singles.tile([128, CODECS_PER_TILE], f32)
        nc.sync.dma_start(blkn_sb, blk_n.ap())
        blkd_sb = singles.tile([128, DYS_PER_TILE], f32)
        nc.sync.dma_start(blkd_sb, blk_d.ap())
        ones_col = singles.tile([128, 1], f32)
        nc.vector.memset(ones_col, 1.0)
        ones_row = singles.tile([1, 128], f32)
        nc.vector.memset(ones_row, 1.0)

        def mlp_embed(m_sb, c, pool, tag, negate=False):
            """means [D, c] (SBUF) -> L2-normalized embeddings [128, 2, c].

            negate=True produces -emb (used as ACT bias in the distance step).
            """
            a1 = pool.tile([128, 4, c], f32, tag=f"{tag}_a1")
            for mc in range(4):
                ps = pmlp.tile([128, c], f32, tag="ps")
                nc.tensor.matmul(ps, w1_sb[:, mc * 128:(mc + 1) * 128], m_sb,
                                 start=True, stop=True)
                nc.scalar.activation(a1[:, mc, :], ps, act.Relu,
                                     bias=b1_sb[:, mc:mc + 1])
            a2 = pool.tile([128, 4, c], f32, tag=f"{tag}_a2")
            for mc in range(4):
                ps = pmlp.tile([128, c], f32, tag="ps")
                for kc in range(4):
                    nc.tensor.matmul(ps, w2_sb[:, kc, mc * 128:(mc + 1) * 128],
                                     a1[:, kc, :], start=(kc == 0), stop=(kc == 3))
                nc.scalar.activation(a2[:, mc, :], ps, act.Relu,
                                     bias=b2_sb[:, mc:mc + 1])
            e3 = pool.tile([128, 2, c], f32, tag=f"{tag}_e3")
            sq = pool.tile([128, 2, c], f32, tag=f"{tag}_sq")
            for ec in range(2):
                ps = pmlp.tile([128, c], f32, tag="ps")
                for kc in range(4):
                    nc.tensor.matmul(ps, w3_sb[:, kc, ec * 128:(ec + 1) * 128],
                                     a2[:, kc, :], start=(kc == 0), stop=(kc == 3))
                nc.scalar.add(e3[:, ec, :], ps, b3_sb[:, ec:ec + 1])
                nc.scalar.square(sq[:, ec, :], e3[:, ec, :])
            n2 = pvec.tile([1, c], f32, tag="n2")
            for ec in range(2):
                nc.tensor.matmul(n2, ones_col, sq[:, ec, :],
                                 start=(ec == 0), stop=(ec == 1))
            nrm = pool.tile([1, c], f32, tag=f"{tag}_nrm")
            nc.scalar.sqrt(nrm, n2)
            rn = pool.tile([1, c], f32, tag=f"{tag}_rn")
            nc.vector.reciprocal(rn, nrm)
            if negate:
                nc.vector.tensor_scalar_mul(rn, rn, -1.0)
            bs = pmlp.tile([128, c], f32, tag="ps")
            nc.tensor.matmul(bs, ones_row, rn, start=True, stop=True)
            nh = pool.tile([128, 2, c], f32, tag=f"{tag}_nh")
            for ec in range(2):
                nc.vector.tensor_mul(nh[:, ec, :], e3[:, ec, :], bs)
            return nh

        def mean_tiles(tiled_view, tile0, ntiles, blk_sb, items_per_tile, dve_tiles):
            """Stream `ntiles` tiles, return psum means [D, ntiles*items_per_tile]."""
            msum = pacc.tile([128, ntiles * items_per_tile], f32, tag="msum")
            for t in range(ntiles):
                st = stream.tile([128, F, D], f32, tag="st")
                nc.sync.dma_start(st, tiled_view[tile0 + t])
                cols = slice(t * items_per_tile, (t + 1) * items_per_tile)
                if t < dve_tiles:
                    # Tree-reduce the 16 f-rows on DVE, then one PE matmul for
                    # the partition-group reduction.
                    h = F // 2
                    while h >= 1:
                        nc.vector.tensor_add(st[:, 0:h, :], st[:, 0:h, :],
                                             st[:, h:2 * h, :])
                        h //= 2
                    nc.tensor.matmul(msum[:, cols], st[:, 0, :], blk_sb,
                                     start=True, stop=True)
                else:
                    for j in range(F):
                        nc.tensor.matmul(msum[:, cols], st[:, j, :], blk_sb,
                                         start=(j == 0), stop=(j == F - 1))
            return msum

        # ---- dysarthric stream (tiny): means -> embeddings ----
        mdsum = mean_tiles(dys_t, 0, B * T // TILE_ROWS, blkd_sb, DYS_PER_TILE, 0)
        md_sb = singles.tile([128, B], f32)
        nc.scalar.copy(md_sb, mdsum)
        # negated normalized query embeddings: dist uses Abs(nh + (-d)) on ACT
        nh_dn = mlp_embed(md_sb, B, singles, "d", negate=True)

        # ---- normal codec groups ----
        for g in range(groups):
            msum = mean_tiles(normal_t, g * group_tiles, group_tiles, blkn_sb,
                              CODECS_PER_TILE, DVE_TILES_PER_GROUP)
            m_sb = sb.tile([128, group_c], f32, tag="m")
            nc.scalar.copy(m_sb, msum)
            nh = mlp_embed(m_sb, group_c, sb, "g")
            for b in range(B):
                pd = pdist.tile([1, group_c], f32, tag="pd")
                for ec in range(2):
                    tmpt = tmp_pool.tile([128, group_c], f32, tag="tmp")
                    nc.scalar.activation(tmpt, nh[:, ec, :], act.Abs,
                                         bias=nh_dn[:, ec, b:b + 1])
                    nc.tensor.matmul(pd, ones_col, tmpt,
                                     start=(ec == 0), stop=(ec == 1))
                ds = dout_pool.tile([1, group_c], f32, tag="ds")
                nc.vector.tensor_copy(ds, pd)
                nc.sync.dma_start(
                    dist.ap()[b:b + 1, g * group_c:(g + 1) * group_c], ds)

    nc.compile()
    return nc


def _get_nc(nsh):
    if nsh not in _BUILD_CACHE:
        _BUILD_CACHE[nsh] = _build(nsh)
    return _BUILD_CACHE[nsh]


def _make_in_maps(dysarthric_codec, normal_codec_set, W1, b1, W2, b2, W3, b3,
                  nsh, n_cores):
    blk_n = np.zeros((128, CODECS_PER_TILE), np.float32)
    ppi_n = 128 // CODECS_PER_TILE
    for i in range(CODECS_PER_TILE):
        blk_n[i * ppi_n:(i + 1) * ppi_n, i] = 1.0 / TN
    blk_d = np.zeros((128, DYS_PER_TILE), np.float32)
    ppi_d = 128 // DYS_PER_TILE
    for i in range(DYS_PER_TILE):
        blk_d[i * ppi_d:(i + 1) * ppi_d, i] = 1.0 / T
    common = {
        "dys": np.ascontiguousarray(dysarthric_codec, np.float32),
        "w1": np.ascontiguousarray(W1, np.float32),
        "b1": np.ascontiguousarray(b1, np.float32),
        "w2": np.ascontiguousarray(W2, np.float32),
        "b2": np.ascontiguousarray(b2, np.float32),
        "w3": np.ascontiguousarray(W3, np.float32),
        "b3": np.ascontiguousarray(b3, np.float32),
        "blk_n": blk_n,
        "blk_d": blk_d,
    }
    in_maps = []
    for k in range(n_cores):
        shard = np.ascontiguousarray(
            normal_codec_set[k * nsh:(k + 1) * nsh], np.float32)
        in_maps.append({**common, "normal": shard})
    return in_maps


def run_device(dysarthric_codec, normal_codec_set, W1, b1, W2, b2, W3, b3,
               trace=False):
    """Run the Bass kernel on the 8 cores, return (dist [B, N], results obj)."""
    from concourse.bass_utils import run_bass_kernel_spmd

    normal_codec_set = np.ascontiguousarray(normal_codec_set, np.float32)
    nc = _get_nc(NSH)
    in_maps = _make_in_maps(dysarthric_codec, normal_codec_set, W1, b1, W2, b2,
                            W3, b3, NSH, N_CORES)
    res = run_bass_kernel_spmd(nc, in_maps, core_ids=list(range(N_CORES)),
                               trace=trace)
    dist_full = np.concatenate([r["dist"] for r in res.results], axis=1)
    return dist_full, res


def kernel(dysarthric_codec, normal_codec_set, W1, b1, W2, b2, W3, b3):
    normal_codec_set = np.ascontiguousarray(np.asarray(normal_codec_set),
                                            np.float32)
    dist_full, _ = run_device(np.asarray(dysarthric_codec), normal_codec_set,
                              np.asarray(W1), np.asarray(b1), np.asarray(W2),
                              np.asarray(b2), np.asarray(W3), np.asarray(b3))
    min_idx = np.argmin(dist_full, axis=1)
    return np.ascontiguousarray(normal_codec_set[min_idx])
